# revision 6
# baseline (speedup 1.0000x reference)
"""CRF NLL loss kernel for 8 Trainium2 NeuronCores — time-sharded forward algorithm.

Math: exp-domain forward recurrence alpha_{s+1} = diag(em_s) M alpha_s with
M = exp(transitions), em prescaled per step by its LSE so fp32/bf16 never
over/underflows. logZ(b) = log(w . alpha_{L_b}) with w = exp(trans[STOP]).

Sharding: TIME-sharded (not batch): each core owns a 128-step range of ALL 512
sequences, split into C=6 chains of 31 steps. A chain's stream starts ~9-11
steps before its owned block; the CRF transfer recurrence contracts initial-
condition error by ~0.2x/step (measured), so after the warmup prefix the state
direction is exact to ~1e-7 and only an unknown per-sequence log-scale remains.
The host stitches those scales chain-to-chain through overlap records. Chain 0
of core 0 starts from the exact one-hot START state (no warmup).

Layout: two 256-sequence groups packed on partitions 0-47 / 48-95 plus two
stop-dot rows (96/97) via a block-diagonal [96,98] weight augmented with the
STOP row, so each step is ONE PE matmul [96,98]x[96,256] (bf16, fp32 psum) and
ONE PSUM->SBUF elementwise multiply by em. The multiply alternates between
three engine paths (fused DVE / fused GPSIMD / Act-copy + 2x-mode DVE mul) to
balance engine busy time; ~18 independent chains per device hide the per-step
cross-engine dependency latency. Records (rows 96/97 of every ring slot) are
DMA'd out; the host converts them to logZ and subtracts the gold path score.
"""
import os
import sys

import numpy as np

for _p in ("/opt/trn_rl_repo", "/root/.axon_site/_ro/trn_rl_repo"):
    if os.path.isdir(_p) and _p not in sys.path:
        sys.path.insert(0, _p)

import ml_dtypes

import concourse.bacc as bacc
import concourse.tile as tile
from concourse import mybir
from concourse import bass_utils

BF16NP = ml_dtypes.bfloat16

B, S, T = 512, 1024, 48
START, STOP, PAD = 45, 46, 47
NCORE = 8
C = 6                    # chains (time blocks) per core
NST = 31                 # steps per chain
NSLOT = NST + 1          # emis slots per chain (init + 31 em steps)
F = 256                  # free dim = sequences per partition-group
P = 98                   # partitions: 48 tags x 2 groups + 2 stop rows
F32 = mybir.dt.float32
BF16 = mybir.dt.bfloat16

# multiply-path schedule: D = fused DVE (psum x em), A = Act copy + 2x-mode
# DVE mul, P = Act copy + GPSIMD mul (GPSIMD cannot read PSUM). Ratios balance
# DVE/Act/Pool busy per the TRN2 cost model.
PAT = "DPDPADPDPAD"

# emis slot s lives at ring col (s - 22 if s >= 22 else s); slots 0..21 occupy
# chunk buffers 0/1, slots 22..31 recycle buffer 0.
CH0, CH1 = 11, 22


def _col_of_slot(s):
    return (s if s < CH1 else s - CH1) * F

_CACHE = {}


def _build_program():
    nc = bacc.Bacc(
        "TRN2",
        target_bir_lowering=False,
        debug=False,
        enable_asserts=False,
        num_devices=NCORE,
    )
    emis_d = nc.dram_tensor("emis", [C, P, NSLOT * F], BF16, kind="ExternalInput").ap()
    w_d = nc.dram_tensor("wts", [96, P], BF16, kind="ExternalInput").ap()
    rec_d = nc.dram_tensor("recs", [C, 2, NST * F], BF16, kind="ExternalOutput").ap()

    with tile.TileContext(nc) as tc:
        with tc.tile_pool(name="main", bufs=1) as pool, tc.tile_pool(
            name="ps", bufs=1, space="PSUM"
        ) as pp:
            wt = pool.tile([96, P], BF16)
            nc.sync.dma_start(out=wt[:, :], in_=w_d[:, :])
            rings = [
                pool.tile([P, NST * F], BF16, tag=f"ring{c}", name=f"ring{c}")
                for c in range(C)
            ]
            ems = [
                pool.tile([P, CH1 * F], BF16, tag=f"em{c}", name=f"em{c}")
                for c in range(C)
            ]
            scr = [
                pool.tile([P, 4 * F], BF16, tag=f"scr{c}", name=f"scr{c}")
                for c in range(C)
            ]
            for c in range(C):
                nc.sync.dma_start(
                    out=ems[c][:, 0 : CH0 * F], in_=emis_d[c, :, 0 : CH0 * F]
                )
            for c in range(C):
                nc.sync.dma_start(
                    out=ems[c][:, CH0 * F : CH1 * F],
                    in_=emis_d[c, :, CH0 * F : CH1 * F],
                )

            for i in range(NST):
                for c in range(C):
                    if i == 10:
                        # buffer 0 (slots 0..10) fully consumed after step 9;
                        # refill with slots 22..31
                        nc.sync.dma_start(
                            out=ems[c][:, 0 : (NSLOT - CH1) * F],
                            in_=emis_d[c, :, CH1 * F : NSLOT * F],
                        )
                    if i == 0:
                        src = ems[c][0:96, 0:F]
                    else:
                        src = rings[c][0:96, (i - 1) * F : i * F]
                    ps = pp.tile([P, F], F32, tag=f"mm{c}")
                    nc.tensor.matmul(ps[:, :], wt[:, :], src, start=True, stop=True)
                    dst = rings[c][:, i * F : (i + 1) * F]
                    emsl = ems[c][:, _col_of_slot(i + 1) : _col_of_slot(i + 1) + F]
                    mv = PAT[(i + 2 * c) % len(PAT)]
                    if mv == "D":
                        nc.vector.tensor_mul(dst, ps[:, :], emsl)
                    else:
                        sc = scr[c][:, (i % 4) * F : (i % 4) * F + F]
                        nc.scalar.copy(sc, ps[:, :])
                        if mv == "P":
                            nc.gpsimd.tensor_mul(dst, sc, emsl)
                        else:
                            nc.vector.tensor_mul(dst, sc, emsl)
                    if i == 15:
                        nc.sync.dma_start(
                            out=rec_d[c, :, 0 : 16 * F],
                            in_=rings[c][96:98, 0 : 16 * F],
                        )
                    elif i == NST - 1:
                        nc.sync.dma_start(
                            out=rec_d[c, :, 16 * F : NST * F],
                            in_=rings[c][96:98, 16 * F : NST * F],
                        )

    nc.compile()
    return nc


def _blocks_for_core(k):
    """(a, t0, t1) per chain: stream = em steps [a, a+31); owned = (t0, t1]."""
    owned = [30, 20, 20, 20, 19, 19] if k == 0 else [22, 22, 21, 21, 21, 21]
    out = []
    t1 = 128 * k
    for o in owned:
        t1 += o
        out.append((t1 - 30, t1 - o, t1))
    return out


def kernel(feats, masks, tags, transitions):
    feats = np.asarray(feats, dtype=np.float32)
    masks = np.asarray(masks, dtype=np.float32)
    tags = np.asarray(tags)
    trans = np.asarray(transitions, dtype=np.float32)

    if "nc" not in _CACHE:
        _CACHE["nc"] = _build_program()
    nc = _CACHE["nc"]

    lengths = masks.sum(1).astype(np.int64)

    # host prescale: em = exp(feats - LSE_tags(feats)); cumulative C added back
    mx = feats.max(2)
    Kp = np.log(np.exp(feats - mx[:, :, None]).sum(2)) + mx
    Cc = np.zeros((B, S + 1), np.float64)
    Cc[:, 1:] = np.cumsum(Kp.astype(np.float64), 1)
    em = np.exp(feats - Kp[:, :, None].astype(np.float32))

    # packed per-step emission pages [S+1, 98, 256] (slot S is a dummy for the
    # one-past-the-end matmul of the last chain)
    base = np.ones((S + 1, P, F), np.float32)
    base[:S, 0:48] = em[0:F].transpose(1, 2, 0)
    base[:S, 48:96] = em[F:B].transpose(1, 2, 0)

    Mexp = np.exp(trans.astype(np.float64))
    w = np.exp(trans[STOP].astype(np.float64))
    W2 = np.zeros((96, P), np.float64)
    W2[0:48, 0:48] = Mexp.T
    W2[48:96, 48:96] = Mexp.T
    W2[0:48, 96] = w
    W2[48:96, 97] = w
    wts = W2.astype(BF16NP)

    init_uni = np.zeros((P, F), np.float32)
    init_uni[0:96] = 1.0
    init_exact = np.zeros((P, F), np.float32)
    init_exact[START] = 1.0
    init_exact[48 + START] = 1.0

    in_maps = []
    for k in range(NCORE):
        emis = np.empty((C, P, NSLOT * F), dtype=BF16NP)
        for c, (a, t0, t1) in enumerate(_blocks_for_core(k)):
            ini = init_exact if (k == 0 and a == 0) else init_uni
            emis[c, :, 0:F] = ini.astype(BF16NP)
            sl = np.ascontiguousarray(
                base[a : a + NST].transpose(1, 0, 2)
            ).reshape(P, NST * F)
            emis[c, :, F:] = sl.astype(BF16NP)
        in_maps.append({"emis": emis, "wts": wts})

    _CACHE["in_maps"] = in_maps
    res = bass_utils.run_bass_kernel_spmd(nc, in_maps, core_ids=list(range(NCORE)))
    results = res.results

    # host: stitch per-chain scale offsets, read logZ at L, subtract gold
    chains = []
    for k in range(NCORE):
        rec = (
            np.asarray(results[k]["recs"])
            .astype(np.float64)
            .reshape(C, 2, NST, F)
        )
        for c, (a, t0, t1) in enumerate(_blocks_for_core(k)):
            chains.append((a, t0, t1, rec[c]))
    chains.sort(key=lambda x: x[2])

    grp = np.arange(B) // F
    lane = np.arange(B) % F

    def logr(rc, t, a):
        return np.log(np.maximum(rc[grp, t - a, lane], 1e-300))

    g_off = np.zeros(B)
    logZ = np.full(B, np.nan)
    prev = None
    for (a, t0, t1, rc) in chains:
        if prev is not None:
            pa, _, _, prc = prev
            lt_prev = logr(prc, t0, pa) + Cc[:, t0] - Cc[:, pa] + g_off
            g_off = lt_prev - (logr(rc, t0, a) + Cc[:, t0] - Cc[:, a])
        sel = (lengths > t0) & (lengths <= t1)
        if sel.any():
            Ls = lengths[sel]
            logZ[sel] = (
                np.log(np.maximum(rc[grp[sel], Ls - a, lane[sel]], 1e-300))
                + Cc[sel, Ls]
                - Cc[sel, a]
                + g_off[sel]
            )
        prev = (a, t0, t1, rc)

    bi = np.arange(B)
    em_g = feats[bi[:, None], np.arange(S)[None, :], tags].astype(np.float64)
    tags_ext = np.concatenate([np.full((B, 1), START, tags.dtype), tags], 1)
    trsc = trans.astype(np.float64)[tags_ext[:, 1:], tags_ext[:, :-1]]
    gold = ((em_g + trsc) * masks.astype(np.float64)).sum(1) + trans[
        STOP, tags_ext[bi, lengths]
    ].astype(np.float64)
    return (logZ - gold).astype(np.float32)
